# revision 15
# baseline (speedup 1.0000x reference)
"""Trainium2 Bass kernel for nn_CrossAttention (B=4, Sx=Sy=2048, D=1024, f32).

Sharding: data-parallel over (batch b, query-half h) -> 8 cores; each core
computes full cross-attention for 1024 query rows of one batch against all
2048 keys of that batch (K/V projections duplicated across the 2 cores
sharing a batch; no collectives).

Per-core pipeline:
  P1: QT[e,s]  = (Wq'^T x xT) + bq'     (Wq' = Wq/sqrt(D), folded on host)
  P2: KT[e,t]  = (Wk^T x yT)  + bk      (scores-ready transposed layout)
  P3: V[t,e]   = yT^T x Wv              (bias bv folded on host, post-gather)
  P4 (per 512-wide s-superblock):
      PT[t,s]  = exp(KT^T @ QT)         (scores^T, softmax numerator)
      out[s,e] = (PT^T @ V) / (PT^T @ ones)

Numerics: projections and scores run in float32r (~TF32 precision at bf16
speed, fp32 PSUM accumulation); the final PV matmul runs in bf16 (softmax
probabilities tolerate it, and halving V/PT frees the SBUF needed to
prefetch every weight load under compute).

All three weight matrices stream through one 3-slot pool of 512-column
halves; slot cycling makes each load overlap the previous phase's compute.
"""

import numpy as np

import concourse.bacc as bacc
import concourse.bass as bass
import concourse.tile as tile
import concourse.mybir as mybir
from concourse.bass_utils import run_bass_kernel_spmd

F32 = mybir.dt.float32
F32R = mybir.dt.float32r
BF16 = mybir.dt.bfloat16

B, SX, SY, D = 4, 2048, 2048, 1024
NCORES = 8
SXH = SX // 2          # query rows per core
DB = D // 128          # contraction blocks
EB = D // 128          # output-feature blocks
TBLK = SY // 128       # key blocks of 128
SSB = 512              # s-superblock width
CH = 256               # activation staging chunk (tokens)

_CACHE = {}


def _build():
    nc = bacc.Bacc("TRN2", target_bir_lowering=False, debug=False,
                   num_devices=NCORES, dynamic_dma_scratch_size=2048)

    xt_d = nc.dram_tensor("xt", [DB, 128, SXH], F32R, kind="ExternalInput").ap()
    yt_d = nc.dram_tensor("yt", [DB, 128, SY], F32R, kind="ExternalInput").ap()
    wq_d = nc.dram_tensor("wq", [DB, 128, D], F32R, kind="ExternalInput").ap()
    wk_d = nc.dram_tensor("wk", [DB, 128, D], F32R, kind="ExternalInput").ap()
    wv_d = nc.dram_tensor("wv", [DB, 128, D], F32R, kind="ExternalInput").ap()
    bq_d = nc.dram_tensor("bq2", [EB, 128], F32, kind="ExternalInput").ap()
    bk_d = nc.dram_tensor("bk2", [EB, 128], F32, kind="ExternalInput").ap()
    out_d = nc.dram_tensor("out", [SXH, D], F32, kind="ExternalOutput").ap()

    with tile.TileContext(nc) as tc:
        with (
            tc.tile_pool(name="misc", bufs=1) as misc,
            tc.tile_pool(name="ostage", bufs=3) as ostage,
            tc.tile_pool(name="rstage", bufs=2) as rstage,
            tc.tile_pool(name="ps_big", bufs=5, space="PSUM") as ps_big,
            tc.tile_pool(name="ps_sum", bufs=2, space="PSUM") as ps_sum,
            tc.tile_pool(name="persist", bufs=1) as persist,
            tc.tile_pool(name="wst", bufs=3) as wst,
            tc.tile_pool(name="ast", bufs=2) as ast,
        ):
            bq_t = misc.tile([128, EB], F32)
            bk_t = misc.tile([128, EB], F32)
            ones_f = misc.tile([128, 2], F32)
            ones_t = misc.tile([128, 2], BF16)
            nc.sync.dma_start(out=bq_t, in_=bq_d.rearrange("eb p -> p eb"))
            nc.sync.dma_start(out=bk_t, in_=bk_d.rearrange("eb p -> p eb"))
            nc.vector.memset(ones_f, 1.0)
            nc.vector.tensor_copy(out=ones_t, in_=ones_f)

            QT = persist.tile([128, EB, SXH], F32R)   # 32 KB/part
            KT = persist.tile([128, EB, SY], F32R)    # 64 KB/part
            V = persist.tile([128, TBLK, D], BF16)    # 32 KB/part
            PT = persist.tile([128, TBLK, SSB], BF16)  # 16 KB/part (per-ssb)

            def w_half(w_dram, eh, pieces=1):
                wt = wst.tile([128, DB, 512], F32R, tag="w")
                step = 512 // pieces
                for q in range(pieces):
                    lo = q * step
                    nc.sync.dma_start(
                        out=wt[:, :, lo:lo + step],
                        in_=w_dram[:, :, eh * 512 + lo:eh * 512 + lo + step]
                        .rearrange("db p e -> p db e"))
                return wt

            # ---- P1: QT[e, s] = Wq'^T @ xT + bq' ----
            # first x-chunk loads ahead of the weights so the DMA queue
            # delivers the first matmul's operands as early as possible
            xtc0 = ast.tile([128, DB, CH], F32R, tag="a")
            nc.sync.dma_start(
                out=xtc0,
                in_=xt_d[:, :, 0:CH].rearrange("db p s -> p db s"))
            wq_h = [w_half(wq_d, 0, pieces=4), w_half(wq_d, 1)]
            for ci in range(SXH // CH):
                s0 = ci * CH
                if ci == 0:
                    xtc = xtc0
                else:
                    xtc = ast.tile([128, DB, CH], F32R, tag="a")
                    nc.sync.dma_start(
                        out=xtc,
                        in_=xt_d[:, :, s0:s0 + CH].rearrange("db p s -> p db s"))
                for eb in range(EB):
                    ps = ps_big.tile([128, CH], F32, tag="ps")
                    for db in range(DB):
                        nc.tensor.matmul(
                            ps,
                            lhsT=wq_h[eb // 4][:, db,
                                               (eb % 4) * 128:(eb % 4 + 1) * 128],
                            rhs=xtc[:, db, :],
                            start=(db == 0), stop=(db == DB - 1))
                    nc.vector.tensor_scalar_add(
                        out=QT[:, eb, s0:s0 + CH], in0=ps,
                        scalar1=bq_t[:, eb:eb + 1])

            # ---- P2: KT[e, t] = Wk^T @ yT + bk ----
            wk_h = [w_half(wk_d, 0), w_half(wk_d, 1)]
            for ci in range(SY // CH):
                t0 = ci * CH
                ytc = ast.tile([128, DB, CH], F32R, tag="a")
                nc.sync.dma_start(
                    out=ytc,
                    in_=yt_d[:, :, t0:t0 + CH].rearrange("db p t -> p db t"))
                for eb in range(EB):
                    ps = ps_big.tile([128, CH], F32, tag="ps")
                    for db in range(DB):
                        nc.tensor.matmul(
                            ps,
                            lhsT=wk_h[eb // 4][:, db,
                                               (eb % 4) * 128:(eb % 4 + 1) * 128],
                            rhs=ytc[:, db, :],
                            start=(db == 0), stop=(db == DB - 1))
                    nc.vector.tensor_scalar_add(
                        out=KT[:, eb, t0:t0 + CH], in0=ps,
                        scalar1=bk_t[:, eb:eb + 1])

            # ---- P3: V[t, e] = yT^T @ Wv ----
            wv_h = [w_half(wv_d, 0), w_half(wv_d, 1)]
            for ci in range(SY // CH):
                t0 = ci * CH
                ytc2 = ast.tile([128, DB, CH], F32R, tag="a")
                nc.sync.dma_start(
                    out=ytc2,
                    in_=yt_d[:, :, t0:t0 + CH].rearrange("db p t -> p db t"))
                for tbi in range(CH // 128):
                    tb = ci * (CH // 128) + tbi
                    for eh in range(D // 512):
                        ps = ps_big.tile([128, 512], F32, tag="ps")
                        for db in range(DB):
                            nc.tensor.matmul(
                                ps,
                                lhsT=ytc2[:, db, tbi * 128:(tbi + 1) * 128],
                                rhs=wv_h[eh][:, db, :],
                                start=(db == 0), stop=(db == DB - 1))
                        nc.vector.tensor_copy(
                            out=V[:, tb, eh * 512:(eh + 1) * 512], in_=ps)

            # ---- P4: attention per s-superblock ----
            for ssb in range(SXH // SSB):
                s0 = ssb * SSB
                for tb in range(TBLK):
                    ps = ps_big.tile([128, SSB], F32, tag="ps")
                    for eb in range(EB):
                        nc.tensor.matmul(
                            ps,
                            lhsT=KT[:, eb, tb * 128:(tb + 1) * 128],
                            rhs=QT[:, eb, s0:s0 + SSB],
                            start=(eb == 0), stop=(eb == EB - 1))
                    nc.scalar.activation(
                        out=PT[:, tb, :], in_=ps,
                        func=mybir.ActivationFunctionType.Exp)
                for sbi in range(SSB // 128):
                    sl = sbi * 128
                    ps0 = ps_big.tile([128, 512], F32, tag="ps")
                    ps1 = ps_big.tile([128, 512], F32, tag="ps")
                    pss = ps_sum.tile([128, 2], F32, tag="pss")
                    for tb in range(TBLK):
                        lhsT = PT[:, tb, sl:sl + 128]
                        nc.tensor.matmul(
                            ps0, lhsT=lhsT, rhs=V[:, tb, 0:512],
                            start=(tb == 0), stop=(tb == TBLK - 1))
                        nc.tensor.matmul(
                            ps1, lhsT=lhsT, rhs=V[:, tb, 512:1024],
                            start=(tb == 0), stop=(tb == TBLK - 1))
                        nc.tensor.matmul(
                            pss, lhsT=lhsT, rhs=ones_t,
                            start=(tb == 0), stop=(tb == TBLK - 1))
                    rec = rstage.tile([128, 1], F32, tag="rec")
                    nc.vector.reciprocal(rec, pss[:, 0:1])
                    for eh in range(2):
                        o = ostage.tile([128, 512], F32, tag="o")
                        nc.vector.tensor_scalar_mul(
                            out=o, in0=(ps0 if eh == 0 else ps1),
                            scalar1=rec[:, 0:1])
                        nc.sync.dma_start(
                            out=out_d[s0 + sl:s0 + sl + 128,
                                      eh * 512:(eh + 1) * 512],
                            in_=o)

    nc.compile()
    return nc


def _get_nc():
    if "nc" not in _CACHE:
        _CACHE["nc"] = _build()
    return _CACHE["nc"]


def make_in_maps(x, y, Wq, bq, Wk, bk, Wv, bv):
    x = np.asarray(x, dtype=np.float32)
    y = np.asarray(y, dtype=np.float32)
    s = np.float32(1.0 / np.sqrt(D))
    wq = np.ascontiguousarray(
        (np.asarray(Wq, dtype=np.float32) * s).reshape(DB, 128, D))
    wk = np.ascontiguousarray(
        np.asarray(Wk, dtype=np.float32).reshape(DB, 128, D))
    wv = np.ascontiguousarray(
        np.asarray(Wv, dtype=np.float32).reshape(DB, 128, D))
    bq2 = np.ascontiguousarray(
        (np.asarray(bq, dtype=np.float32) * s).reshape(EB, 128))
    bk2 = np.ascontiguousarray(
        np.asarray(bk, dtype=np.float32).reshape(EB, 128))

    in_maps = []
    for c in range(NCORES):
        b, h = divmod(c, 2)
        xt = np.ascontiguousarray(
            x[b, h * SXH:(h + 1) * SXH, :].T).reshape(DB, 128, SXH)
        yt = np.ascontiguousarray(y[b].T).reshape(DB, 128, SY)
        in_maps.append({
            "xt": xt, "yt": yt, "wq": wq, "wk": wk, "wv": wv,
            "bq2": bq2, "bk2": bk2,
        })
    return in_maps


def assemble(results, bv):
    bv = np.asarray(bv, dtype=np.float32)
    out = np.empty((B, SX, D), dtype=np.float32)
    for c in range(NCORES):
        b, h = divmod(c, 2)
        out[b, h * SXH:(h + 1) * SXH, :] = results[c]["out"]
    out += bv[None, None, :]
    return out


def kernel(x, y, Wq, bq, Wk, bk, Wv, bv):
    nc = _get_nc()
    in_maps = make_in_maps(x, y, Wq, bq, Wk, bk, Wv, bv)
    res = run_bass_kernel_spmd(nc, in_maps, list(range(NCORES)))
    return assemble(res.results, bv)
